# revision 7
# baseline (speedup 1.0000x reference)
# Bass/Tile kernel for nn_LongTermAttention (continuous long-term attention
# with rectangular basis functions) on 8 Trainium2 NeuronCores.
#
# Mathematical rewrite (verified exact vs the reference):
#   * G = F^T (F F^T + ridge I)^{-1} for the rectangular basis on the padded
#     uniform grid collapses to G[l, n] = (1/4.5) * [l // 4 == n], so
#     Bc[b,n,e] = (1/4.5) * sum_{j<4} k[b,e,4n+j]  (4-wide sum pooling).
#   * psi on the integration grid is a one-hot selector, so the P=1000-point
#     continuous softmax reduces to basis space:
#       u_n   = exp(s_n)                      (|s| <= ~3, exp safe)
#       Z     = sum_n Wn_n u_n + w_last       (Wn = quadrature mass per basis)
#       ctx   = (u / Z) @ (Wn * values)
#     The max-subtraction in the reference cancels exactly.
#
# v2 performance structure:
#   * k is re-laid-out on host as kj[b, j, e, n] = k[b, e, 4n+j]; the 4-wide
#     pooling then happens INSIDE the DMA via SWDGE accum_op=add (4
#     accumulating transfers land k directly as pooled [e, n] tiles).
#     This removes all vector/gpsimd pooling work from the old design.
#   * q is transposed on host to qT[b, e, t] (no device/DMA transposes).
#   * exp is done in 2 big ACTIVATEs per (batch, head-pair) with no bias;
#     the quadrature mass Wn is folded into the values drain (a
#     tensor_scalar_mul that replaces the plain PSUM-drain copy) and the
#     Z column of values.
#   * ctx for one (batch, head-pair) accumulates into a single PSUM bank
#     [128, 4*65]; Z-normalization is batched (one add + one reciprocal
#     per head-pair, per-chain drain-muls split across vector/gpsimd).
#   * ~7 dummy warm-up matmuls at t=0 keep the PE HAM busy while the first
#     k tiles stream in, so real matmuls run at 2.4 GHz.
#
# Sharding: data-parallel over batch, 2 batches per core; weights replicated.

import numpy as np

B_FULL = 16
N_CORES = 8
B_PER = B_FULL // N_CORES  # 2
E = 512          # embed dim
L = 2048         # memory length
T = 256          # query length
N = 512          # basis count
H = 8            # heads
D = 64           # head dim
P_GRID = 1000    # integration points
RIDGE_C = 4.5    # F F^T diag (4.0) + ridge (0.5)
W_LAST = 1.0 / 1998.0

N_WARMUP_MM = 7

_CACHE = {}


def _host_constants(Wk, Wv):
    """Fold pooling normalization (1/4.5) and query scale (1/8) into the
    projection weights; build the per-basis quadrature-mass tile."""
    import ml_dtypes
    wk = (Wk.astype(np.float64) / (RIDGE_C * 8.0)).astype(ml_dtypes.bfloat16)
    wv = (Wv.astype(np.float64) / RIDGE_C).astype(ml_dtypes.bfloat16)
    p = np.arange(P_GRID)
    nmap = (512 * p) // 999
    w = np.full(P_GRID, 1.0 / 999.0)
    w[0] = w[-1] = 1.0 / 1998.0
    Wn = np.zeros(N)
    for i in range(P_GRID - 1):
        Wn[nmap[i]] += w[i]
    # win8[p, m*8 + h] = Wn[m*128 + p]  (8 identical cols per n-block m)
    win8 = np.repeat(Wn.reshape(4, 128, 1), 8, axis=2)  # [4, 128, 8]
    win8 = np.ascontiguousarray(win8.transpose(1, 0, 2).reshape(128, 32)
                                ).astype(np.float32)
    return wk, wv, win8


def _build_program():
    import concourse.bass as bass
    import concourse.mybir as mybir
    import concourse.tile as tile
    from concourse import bacc

    f32 = mybir.dt.float32
    bf16 = mybir.dt.bfloat16

    nc = bacc.Bacc(
        "TRN2",
        target_bir_lowering=False,
        debug=False,
        enable_asserts=False,
        num_devices=N_CORES,
    )

    kj_d = nc.dram_tensor("kj", [B_PER, 4, E, N], bf16, kind="ExternalInput").ap()
    qt_d = nc.dram_tensor("qt", [B_PER, E, T], bf16, kind="ExternalInput").ap()
    wk_d = nc.dram_tensor("wk", [E, E], bf16, kind="ExternalInput").ap()
    wv_d = nc.dram_tensor("wv", [E, E], bf16, kind="ExternalInput").ap()
    win8_d = nc.dram_tensor("win8", [128, 32], f32, kind="ExternalInput").ap()
    out_d = nc.dram_tensor("out", [B_PER, T, E], bf16, kind="ExternalOutput").ap()

    from contextlib import ExitStack
    with tile.TileContext(nc) as tc, ExitStack() as ctx:
        _kernel_body(ctx, tc, nc, mybir, kj_d, qt_d, wk_d, wv_d, win8_d, out_d)

    nc.compile()
    return nc


def _kernel_body(ctx, tc, nc, mybir, kj_d, qt_d, wk_d, wv_d, win8_d, out_d):
    f32 = mybir.dt.float32
    bf16 = mybir.dt.bfloat16
    Exp = mybir.ActivationFunctionType.Exp
    ADD = mybir.AluOpType.add

    def pool(name, bufs, space="SBUF"):
        return ctx.enter_context(tc.tile_pool(name=name, bufs=bufs, space=space))

    consts = pool("consts", 1)
    plpool = pool("plpool", 8)
    qtpool = pool("qtpool", 8)
    ktpool = pool("ktpool", 8)
    vpool = pool("vpool", 8)
    upool = pool("upool", 4)
    zpool = pool("zpool", 8)
    opool = pool("opool", 4)

    ps_proj = pool("ps_proj", 2, "PSUM")   # [128,512] : 1 bank each
    ps_s = pool("ps_s", 2, "PSUM")         # [128,1024]: 2 banks each
    ps_c = pool("ps_c", 2, "PSUM")         # [128,260] : 1 bank each

    # ---- constants on HWDGE rings ----
    wk_sb = consts.tile([128, 4 * 512], bf16, tag="wk")   # [e%128, (e//128)*512 + e']
    wv_sb = consts.tile([128, 4 * 512], bf16, tag="wv")
    nc.sync.dma_start(wk_sb[:].rearrange("p (kk e) -> p kk e", kk=4),
                      wk_d.rearrange("(kk p) e -> p kk e", p=128))
    nc.scalar.dma_start(wv_sb[:].rearrange("p (kk e) -> p kk e", kk=4),
                        wv_d.rearrange("(kk p) e -> p kk e", p=128))
    win8_sb = consts.tile([128, 32], f32, tag="win8")
    nc.sync.dma_start(win8_sb[:], win8_d[:])

    # ---- qT straight loads (host pre-transposed) ----
    qT_b = []
    for b in range(B_PER):
        qT = []
        for eb in range(4):
            qt_sb = qtpool.tile([128, T], bf16, tag="qT")
            eng = nc.sync if eb % 2 == 0 else nc.scalar
            eng.dma_start(qt_sb[:], qt_d[b, eb * 128:(eb + 1) * 128, :])
            qT.append(qt_sb)
        qT_b.append(qT)

    # ---- PE warm-up: dummy matmuls with no deps keep HAM busy from t=0 ----
    dummy = consts.tile([128, 512], bf16, tag="dummy")
    nc.vector.memset(dummy[:], 0.125)
    ps_w = ps_s.tile([128, 1024], f32, tag="ps_s")
    for i in range(N_WARMUP_MM):
        nc.tensor.matmul(ps_w[:, 0:512], dummy[:, 0:128], dummy[:],
                         start=True, stop=True, skip_group_check=True)

    # ---- k DMA with in-flight pooling (SWDGE accumulate) ----
    # kj[b, j, e, n] = k[b, e, 4n+j]; 4 transfers accumulate into pooled[b][kk].
    # Tile pairs (kk, kk+1) are interleaved so the write-after-write wait of
    # one tile's next accumulate overlaps the other tile's transfer.
    pooled_b = [[plpool.tile([128, N], bf16, tag="pl", name=f"pl{b}_{kk}")
                 for kk in range(4)] for b in range(B_PER)]
    for b in range(B_PER):
        for pair in ((0, 1), (2, 3)):
            for j in range(4):
                for kk in pair:
                    nc.gpsimd.dma_start(
                        pooled_b[b][kk][:],
                        kj_d[b, j, kk * 128:(kk + 1) * 128, :],
                        accum_op=(mybir.AluOpType.bypass if j == 0 else ADD),
                    )

    for b in range(B_PER):
        pooled = pooled_b[b]
        qT = qT_b[b]

        # ---- keysT = wk^T @ pooled  -> [e' (4x128 part), n=512] ----
        keysT = []
        for m in range(4):
            ps = ps_proj.tile([128, 512], f32, tag="ps_proj")
            for kk in range(4):
                nc.tensor.matmul(
                    ps[:],
                    wk_sb[:, kk * 512 + m * 128: kk * 512 + (m + 1) * 128],
                    pooled[kk][:],
                    start=(kk == 0), stop=(kk == 3),
                )
            kt_sb = ktpool.tile([128, 512], bf16, tag="keysT")
            nc.vector.tensor_copy(kt_sb[:], ps[:])
            keysT.append(kt_sb)

        # ---- values' = Wn * (pooled^T @ wv) -> [n (4x128 part), 8*(64+Wn)] ----
        values = []
        for m in range(4):
            ps = ps_proj.tile([128, 512], f32, tag="ps_proj")
            for kk in range(4):
                nc.tensor.matmul(
                    ps[:],
                    pooled[kk][:, m * 128:(m + 1) * 128],
                    wv_sb[:, kk * 512:(kk + 1) * 512],
                    start=(kk == 0), stop=(kk == 3),
                )
            v_sb = vpool.tile([128, 8 * 66], bf16, tag="values")
            vv = v_sb[:].rearrange("p (h c) -> p h c", c=66)
            nc.vector.tensor_scalar_mul(
                vv[:, :, 0:64],
                ps[:].rearrange("p (h d) -> p h d", d=64),
                win8_sb[:, m * 8:m * 8 + 1],
            )
            nc.gpsimd.tensor_copy(vv[:, :, 64], win8_sb[:, m * 8:(m + 1) * 8])
            values.append(v_sb)

        # ---- per head pair: scores -> exp -> ctx -> normalized out ----
        out_sbs = [opool.tile([128, E], bf16, tag="out", name=f"out{b}_{mb}")
                   for mb in range(2)]
        for hp in range(4):
            # scores: per h01 one [128, 4nb*256t] PSUM tile (2 banks);
            # the h01 pair runs concurrently via PE row tiling (base
            # partitions 0/64) and drains to different banks.
            ps_h = [ps_s.tile([128, 1024], f32, tag="ps_s",
                              name=f"ps_s{b}_{hp}_{h01}") for h01 in range(2)]
            for nb in range(4):
                for h01 in range(2):
                    nc.tensor.matmul(
                        ps_h[h01][:, nb * 256:(nb + 1) * 256],
                        keysT[hp][h01 * 64:(h01 + 1) * 64,
                                  nb * 128:(nb + 1) * 128],
                        qT[hp][h01 * 64:(h01 + 1) * 64, :],
                        start=True, stop=True,
                        skip_group_check=True,
                    )
            # exp: one big ACTIVATE per h01 (no bias; Wn lives in values')
            u_h = []
            for h01 in range(2):
                u = upool.tile([128, 1024], bf16, tag="u")
                nc.scalar.activation(u[:], ps_h[h01][:], Exp)
                u_h.append(u)

            # ctx (+Z at col 64 of each 65-block): 4 chains into one bank
            psc = ps_c.tile([128, 260], f32, tag="ps_c")
            for h01 in range(2):
                h = hp * 2 + h01
                for mb in range(2):
                    c = (2 * h01 + mb) * 65
                    for nb in range(4):
                        nc.tensor.matmul(
                            psc[:, c:c + 65],
                            u_h[h01][:, nb * 256 + mb * 128:
                                     nb * 256 + (mb + 1) * 128],
                            values[nb][:, h * 66:h * 66 + 65],
                            start=(nb == 0), stop=(nb == 3),
                            skip_group_check=True,
                        )
            # batched Z normalization: drain the whole head-pair PSUM to an
            # SBUF staging tile (vector), then normalize on gpsimd
            # (gpsimd cannot touch PSUM).
            pv = psc[:].rearrange("p (g c) -> p g c", c=65)
            stage = zpool.tile([128, 260], f32, tag="stage")
            nc.vector.tensor_copy(stage[:], psc[:])
            sv = stage[:].rearrange("p (g c) -> p g c", c=65)
            z = zpool.tile([128, 4], f32, tag="z")
            nc.vector.tensor_scalar_add(z[:], pv[:, :, 64], W_LAST)
            zi = zpool.tile([128, 4], f32, tag="zi")
            nc.vector.reciprocal(zi[:], z[:])
            for h01 in range(2):
                h = hp * 2 + h01
                for mb in range(2):
                    idx = 2 * h01 + mb
                    nc.gpsimd.tensor_scalar_mul(
                        out_sbs[mb][:, h * 64:(h + 1) * 64],
                        sv[:, idx, 0:64],
                        zi[:, idx:idx + 1],
                    )
        nc.sync.dma_start(out_d[b, 0:128, :], out_sbs[0][:])
        nc.scalar.dma_start(out_d[b, 128:256, :], out_sbs[1][:])


def _get_program():
    if "nc" not in _CACHE:
        _CACHE["nc"] = _build_program()
    return _CACHE["nc"]


def make_in_maps(k, q, Wk, Wv):
    import ml_dtypes
    wk, wv, win8 = _host_constants(Wk, Wv)
    k16 = np.asarray(k).astype(ml_dtypes.bfloat16)
    # kj[b, j, e, n] = k[b, e, 4n+j]
    kj = np.ascontiguousarray(
        k16.reshape(B_FULL, E, N, 4).transpose(0, 3, 1, 2))
    qt = np.ascontiguousarray(
        np.asarray(q).astype(ml_dtypes.bfloat16).transpose(0, 2, 1))
    in_maps = []
    for c in range(N_CORES):
        in_maps.append({
            "kj": np.ascontiguousarray(kj[c * B_PER:(c + 1) * B_PER]),
            "qt": np.ascontiguousarray(qt[c * B_PER:(c + 1) * B_PER]),
            "wk": wk,
            "wv": wv,
            "win8": win8,
        })
    return in_maps


def kernel(k, q, Wk, Wv):
    from concourse.bass_utils import run_bass_kernel_spmd

    in_maps = make_in_maps(k, q, Wk, Wv)
    nc = _get_program()
    res = run_bass_kernel_spmd(nc, in_maps, core_ids=list(range(N_CORES)))
    return np.concatenate(
        [res.results[c]["out"].astype(np.float32) for c in range(N_CORES)],
        axis=0)


# revision 15
# speedup vs baseline: 1.4437x; 1.4437x over previous
# Bass/Tile kernel for nn_LongTermAttention (continuous long-term attention
# with rectangular basis functions) on 8 Trainium2 NeuronCores.
#
# Mathematical rewrite (verified exact vs the reference):
#   * G = F^T (F F^T + ridge I)^{-1} for the rectangular basis on the padded
#     uniform grid collapses to G[l, n] = (1/4.5) * [l // 4 == n], so
#     Bc[b,n,e] = (1/4.5) * sum_{j<4} k[b,e,4n+j]  (4-wide sum pooling).
#   * psi on the integration grid is a one-hot selector, so the P=1000-point
#     continuous softmax reduces to basis space:
#       u_n   = exp(s_n)                      (|s| <= ~3, exp safe)
#       Z     = sum_n Wn_n u_n + w_last       (Wn = quadrature mass per basis)
#       ctx   = (u / Z) @ (Wn * values)
#     The max-subtraction in the reference cancels exactly.
#
# v2 performance structure:
#   * k is re-laid-out on host as kj[b, j, e, n] = k[b, e, 4n+j]; the 4-wide
#     pooling then happens INSIDE the DMA via SWDGE accum_op=add (4
#     accumulating transfers land k directly as pooled [e, n] tiles).
#     This removes all vector/gpsimd pooling work from the old design.
#   * q is transposed on host to qT[b, e, t] (no device/DMA transposes).
#   * exp is done in 2 big ACTIVATEs per (batch, head-pair) with no bias;
#     the quadrature mass Wn is folded into the values drain (a
#     tensor_scalar_mul that replaces the plain PSUM-drain copy) and the
#     Z column of values.
#   * ctx for one (batch, head-pair) accumulates into a single PSUM bank
#     [128, 4*65]; Z-normalization is batched (one add + one reciprocal
#     per head-pair, per-chain drain-muls split across vector/gpsimd).
#   * ~7 dummy warm-up matmuls at t=0 keep the PE HAM busy while the first
#     k tiles stream in, so real matmuls run at 2.4 GHz.
#
# Sharding: data-parallel over batch, 2 batches per core; weights replicated.

import numpy as np

B_FULL = 16
N_CORES = 8
B_PER = B_FULL // N_CORES  # 2
E = 512          # embed dim
L = 2048         # memory length
T = 256          # query length
N = 512          # basis count
H = 8            # heads
D = 64           # head dim
P_GRID = 1000    # integration points
RIDGE_C = 4.5    # F F^T diag (4.0) + ridge (0.5)
W_LAST = 1.0 / 1998.0

N_WARMUP_MM = 7

_CACHE = {}


def _host_constants(Wk, Wv):
    """Fold pooling normalization (1/4.5) and query scale (1/8) into the
    projection weights; build the per-basis quadrature-mass tile."""
    import ml_dtypes
    wk = (Wk.astype(np.float64) / (RIDGE_C * 8.0)).astype(ml_dtypes.bfloat16)
    wv = (Wv.astype(np.float64) / RIDGE_C).astype(ml_dtypes.bfloat16)
    p = np.arange(P_GRID)
    nmap = (512 * p) // 999
    w = np.full(P_GRID, 1.0 / 999.0)
    w[0] = w[-1] = 1.0 / 1998.0
    Wn = np.zeros(N)
    for i in range(P_GRID - 1):
        Wn[nmap[i]] += w[i]
    # win8[p, m*8 + h] = Wn[m*128 + p]  (8 identical cols per n-block m)
    win8 = np.repeat(Wn.reshape(4, 128, 1), 8, axis=2)  # [4, 128, 8]
    win8 = np.ascontiguousarray(win8.transpose(1, 0, 2).reshape(128, 32)
                                ).astype(np.float32)
    return wk, wv, win8


def _build_program():
    import concourse.bass as bass
    import concourse.mybir as mybir
    import concourse.tile as tile
    from concourse import bacc

    f32 = mybir.dt.float32
    bf16 = mybir.dt.bfloat16

    nc = bacc.Bacc(
        "TRN2",
        target_bir_lowering=False,
        debug=False,
        enable_asserts=False,
        num_devices=N_CORES,
    )

    kp_d = nc.dram_tensor("kp", [B_PER, E, L], bf16, kind="ExternalInput").ap()
    qt_d = nc.dram_tensor("qt", [B_PER, E, T], bf16, kind="ExternalInput").ap()
    wk_d = nc.dram_tensor("wk", [E, E], bf16, kind="ExternalInput").ap()
    wv_d = nc.dram_tensor("wv", [E, E], bf16, kind="ExternalInput").ap()
    win8_d = nc.dram_tensor("win8", [128, 32], f32, kind="ExternalInput").ap()
    out_d = nc.dram_tensor("out", [B_PER, T, E], bf16, kind="ExternalOutput").ap()

    from contextlib import ExitStack
    with tile.TileContext(nc) as tc, ExitStack() as ctx:
        _kernel_body(ctx, tc, nc, mybir, kp_d, qt_d, wk_d, wv_d, win8_d, out_d)

    nc.compile()
    return nc


def _kernel_body(ctx, tc, nc, mybir, kp_d, qt_d, wk_d, wv_d, win8_d, out_d):
    f32 = mybir.dt.float32
    bf16 = mybir.dt.bfloat16
    Exp = mybir.ActivationFunctionType.Exp

    def pool(name, bufs, space="SBUF"):
        return ctx.enter_context(tc.tile_pool(name=name, bufs=bufs, space=space))

    consts = pool("consts", 1)
    kpool = pool("kpool", 6)
    t1pool = pool("t1pool", 4)
    plpool = pool("plpool", 8)
    qtpool = pool("qtpool", 8)
    ktpool = pool("ktpool", 8)
    vpool = pool("vpool", 8)
    upool = pool("upool", 4)
    zpool = pool("zpool", 8)
    opool = pool("opool", 4)

    ps_proj = pool("ps_proj", 2, "PSUM")   # [128,512] : 1 bank each
    ps_s = pool("ps_s", 2, "PSUM")         # [128,1024]: 2 banks each
    ps_c = pool("ps_c", 2, "PSUM")         # [128,260] : 1 bank each

    # ---- constants on HWDGE rings ----
    wk_sb = consts.tile([128, 4 * 512], bf16, tag="wk")   # [e%128, (e//128)*512 + e']
    wv_sb = consts.tile([128, 4 * 512], bf16, tag="wv")
    nc.sync.dma_start(wk_sb[:].rearrange("p (kk e) -> p kk e", kk=4),
                      wk_d.rearrange("(kk p) e -> p kk e", p=128))
    nc.scalar.dma_start(wv_sb[:].rearrange("p (kk e) -> p kk e", kk=4),
                        wv_d.rearrange("(kk p) e -> p kk e", p=128))
    win8_sb = consts.tile([128, 32], f32, tag="win8")
    nc.sync.dma_start(win8_sb[:], win8_d[:])

    # ---- qT straight loads (host pre-transposed) ----
    qT_b = []
    for b in range(B_PER):
        qT = []
        for eb in range(4):
            qt_sb = qtpool.tile([128, T], bf16, tag="qT")
            eng = nc.sync if eb % 2 == 0 else nc.scalar
            eng.dma_start(qt_sb[:], qt_d[b, eb * 128:(eb + 1) * 128, :])
            qT.append(qt_sb)
        qT_b.append(qT)

    # ---- PE warm-up: dummy matmuls with no deps keep HAM busy from t=0 ----
    dummy = consts.tile([128, 512], bf16, tag="dummy")
    nc.vector.memset(dummy[:], 0.125)
    ps_w = ps_s.tile([128, 1024], f32, tag="ps_s")
    for i in range(N_WARMUP_MM):
        nc.tensor.matmul(ps_w[:, 0:512], dummy[:, 0:128], dummy[:],
                         start=True, stop=True, skip_group_check=True)

    # ---- k DMA + pooling ----
    # kp is host-permuted so that both pooling stages are contiguous-half
    # adds: kp[b, e, i] = k[b, e, 4*(i%512) + i//512], hence
    #   t1[c]     = kp[c] + kp[c+1024]
    #   pooled[n] = t1[n] + t1[n+512]  ==  sum_j k[e, 4n+j]
    # Contiguous operands let the DVE run its fast path (strided pair-adds
    # measured ~2x slower).  Vector pools e-blocks 0/1, gpsimd 2/3.
    pooled_b = [[plpool.tile([128, N], bf16, tag="pl", name=f"pl{b}_{kk}")
                 for kk in range(4)] for b in range(B_PER)]
    for b in range(B_PER):
        for et in range(4):
            kt = kpool.tile([128, L], bf16, tag="k", name=f"k{b}_{et}")
            dma_eng = nc.sync if et % 2 == 0 else nc.scalar
            dma_eng.dma_start(kt[:], kp_d[b, et * 128:(et + 1) * 128, :])
            eng = nc.vector if et < 2 else nc.gpsimd
            t1 = t1pool.tile([128, L // 2], bf16,
                             tag="t1v" if et < 2 else "t1g",
                             name=f"t1_{b}_{et}")
            eng.tensor_add(t1[:], kt[:, 0:1024], kt[:, 1024:2048])
            eng.tensor_add(pooled_b[b][et][:], t1[:, 0:512], t1[:, 512:1024])

    for b in range(B_PER):
        pooled = pooled_b[b]
        qT = qT_b[b]

        # ---- keysT = wk^T @ pooled  -> [e' (4x128 part), n=512] ----
        keysT = []
        for m in range(4):
            ps = ps_proj.tile([128, 512], f32, tag="ps_proj")
            for kk in range(4):
                nc.tensor.matmul(
                    ps[:],
                    wk_sb[:, kk * 512 + m * 128: kk * 512 + (m + 1) * 128],
                    pooled[kk][:],
                    start=(kk == 0), stop=(kk == 3),
                )
            kt_sb = ktpool.tile([128, 512], bf16, tag="keysT")
            # b0 drains fit in scalar's pre-exp idle window; b1 on vector
            if b == 0:
                nc.scalar.copy(kt_sb[:], ps[:])
            else:
                nc.vector.tensor_copy(kt_sb[:], ps[:])
            keysT.append(kt_sb)

        # ---- values' = Wn * (pooled^T @ wv) -> [n (4x128 part), 8*(64+Wn)] ----
        values = []
        for m in range(4):
            ps = ps_proj.tile([128, 512], f32, tag="ps_proj")
            for kk in range(4):
                nc.tensor.matmul(
                    ps[:],
                    pooled[kk][:, m * 128:(m + 1) * 128],
                    wv_sb[:, kk * 512:(kk + 1) * 512],
                    start=(kk == 0), stop=(kk == 3),
                )
            v_sb = vpool.tile([128, 8 * 66], bf16, tag="values")
            vv = v_sb[:].rearrange("p (h c) -> p h c", c=66)
            if b == 0:
                nc.scalar.activation(
                    vv[:, :, 0:64],
                    ps[:].rearrange("p (h d) -> p h d", d=64),
                    mybir.ActivationFunctionType.Copy,
                    scale=win8_sb[:, m * 8:m * 8 + 1],
                )
            else:
                nc.vector.tensor_scalar_mul(
                    vv[:, :, 0:64],
                    ps[:].rearrange("p (h d) -> p h d", d=64),
                    win8_sb[:, m * 8:m * 8 + 1],
                )
            nc.vector.tensor_copy(vv[:, :, 64], win8_sb[:, m * 8:(m + 1) * 8])
            values.append(v_sb)

        # ---- per head pair: scores -> exp -> ctx -> normalized out ----
        out_sbs = [opool.tile([128, E], bf16, tag="out", name=f"out{b}_{mb}")
                   for mb in range(2)]
        for hp in range(4):
            # scores: per h01 one [128, 4nb*256t] PSUM tile (2 banks);
            # the h01 pair runs concurrently via PE row tiling (base
            # partitions 0/64) and drains to different banks.
            ps_h = [ps_s.tile([128, 1024], f32, tag="ps_s",
                              name=f"ps_s{b}_{hp}_{h01}") for h01 in range(2)]
            for nb in range(4):
                for h01 in range(2):
                    nc.tensor.matmul(
                        ps_h[h01][:, nb * 256:(nb + 1) * 256],
                        keysT[hp][h01 * 64:(h01 + 1) * 64,
                                  nb * 128:(nb + 1) * 128],
                        qT[hp][h01 * 64:(h01 + 1) * 64, :],
                        start=True, stop=True,
                        skip_group_check=True,
                    )
            # exp: one big ACTIVATE per h01 (no bias; Wn lives in values')
            u_h = []
            for h01 in range(2):
                u = upool.tile([128, 1024], bf16, tag="u")
                nc.scalar.activation(u[:], ps_h[h01][:], Exp)
                u_h.append(u)

            # ctx (+Z at col 64 of each 65-block): 4 chains into one bank
            psc = ps_c.tile([128, 260], f32, tag="ps_c")
            for h01 in range(2):
                h = hp * 2 + h01
                for mb in range(2):
                    c = (2 * h01 + mb) * 65
                    for nb in range(4):
                        nc.tensor.matmul(
                            psc[:, c:c + 65],
                            u_h[h01][:, nb * 256 + mb * 128:
                                     nb * 256 + (mb + 1) * 128],
                            values[nb][:, h * 66:h * 66 + 65],
                            start=(nb == 0), stop=(nb == 3),
                            skip_group_check=True,
                        )
            # batched Z normalization for the whole head pair; drain-muls
            # read PSUM directly (vector, one on scalar for balance)
            pv = psc[:].rearrange("p (g c) -> p g c", c=65)
            z = zpool.tile([128, 4], f32, tag="z")
            nc.vector.tensor_scalar_add(z[:], pv[:, :, 64], W_LAST)
            zi = zpool.tile([128, 4], f32, tag="zi")
            nc.vector.reciprocal(zi[:], z[:])
            for h01 in range(2):
                h = hp * 2 + h01
                for mb in range(2):
                    idx = 2 * h01 + mb
                    if h01 == 1 and mb == 1:
                        nc.scalar.activation(
                            out_sbs[mb][:, h * 64:(h + 1) * 64],
                            pv[:, idx, 0:64],
                            mybir.ActivationFunctionType.Copy,
                            scale=zi[:, idx:idx + 1],
                        )
                    else:
                        nc.vector.tensor_scalar_mul(
                            out_sbs[mb][:, h * 64:(h + 1) * 64],
                            pv[:, idx, 0:64],
                            zi[:, idx:idx + 1],
                        )
        nc.sync.dma_start(out_d[b, 0:128, :], out_sbs[0][:])
        nc.scalar.dma_start(out_d[b, 128:256, :], out_sbs[1][:])


def _get_program():
    if "nc" not in _CACHE:
        _CACHE["nc"] = _build_program()
    return _CACHE["nc"]


def make_in_maps(k, q, Wk, Wv):
    import ml_dtypes
    wk, wv, win8 = _host_constants(Wk, Wv)
    k16 = np.asarray(k).astype(ml_dtypes.bfloat16)
    # kp[b, e, i] = k[b, e, 4*(i%512) + i//512]  (makes both pooling
    # stages contiguous-half adds on device)
    kp = np.ascontiguousarray(
        k16.reshape(B_FULL, E, N, 4).transpose(0, 1, 3, 2).reshape(B_FULL, E, L))
    qt = np.ascontiguousarray(
        np.asarray(q).astype(ml_dtypes.bfloat16).transpose(0, 2, 1))
    in_maps = []
    for c in range(N_CORES):
        in_maps.append({
            "kp": np.ascontiguousarray(kp[c * B_PER:(c + 1) * B_PER]),
            "qt": np.ascontiguousarray(qt[c * B_PER:(c + 1) * B_PER]),
            "wk": wk,
            "wv": wv,
            "win8": win8,
        })
    return in_maps


def kernel(k, q, Wk, Wv):
    from concourse.bass_utils import run_bass_kernel_spmd

    in_maps = make_in_maps(k, q, Wk, Wv)
    nc = _get_program()
    res = run_bass_kernel_spmd(nc, in_maps, core_ids=list(range(N_CORES)))
    return np.concatenate(
        [res.results[c]["out"].astype(np.float32) for c in range(N_CORES)],
        axis=0)


# revision 17
# speedup vs baseline: 1.5655x; 1.0844x over previous
# Bass/Tile kernel for nn_LongTermAttention (continuous long-term attention
# with rectangular basis functions) on 8 Trainium2 NeuronCores.
#
# Mathematical rewrite (verified exact vs the reference):
#   * G = F^T (F F^T + ridge I)^{-1} for the rectangular basis on the padded
#     uniform grid collapses to G[l, n] = (1/4.5) * [l // 4 == n], so
#     Bc[b,n,e] = (1/4.5) * sum_{j<4} k[b,e,4n+j]  (4-wide sum pooling).
#   * psi on the integration grid is a one-hot selector, so the P=1000-point
#     continuous softmax reduces to basis space:
#       u_n   = exp(s_n)                      (|s| <= ~3, exp safe)
#       Z     = sum_n Wn_n u_n + w_last       (Wn = quadrature mass per basis)
#       ctx   = (u / Z) @ (Wn * values)
#     The max-subtraction in the reference cancels exactly.
#
# v2 performance structure:
#   * k is re-laid-out on host as kj[b, j, e, n] = k[b, e, 4n+j]; the 4-wide
#     pooling then happens INSIDE the DMA via SWDGE accum_op=add (4
#     accumulating transfers land k directly as pooled [e, n] tiles).
#     This removes all vector/gpsimd pooling work from the old design.
#   * q is transposed on host to qT[b, e, t] (no device/DMA transposes).
#   * exp is done in 2 big ACTIVATEs per (batch, head-pair) with no bias;
#     the quadrature mass Wn is folded into the values drain (a
#     tensor_scalar_mul that replaces the plain PSUM-drain copy) and the
#     Z column of values.
#   * ctx for one (batch, head-pair) accumulates into a single PSUM bank
#     [128, 4*65]; Z-normalization is batched (one add + one reciprocal
#     per head-pair, per-chain drain-muls split across vector/gpsimd).
#   * ~7 dummy warm-up matmuls at t=0 keep the PE HAM busy while the first
#     k tiles stream in, so real matmuls run at 2.4 GHz.
#
# Sharding: data-parallel over batch, 2 batches per core; weights replicated.

import numpy as np

B_FULL = 16
N_CORES = 8
B_PER = B_FULL // N_CORES  # 2
E = 512          # embed dim
L = 2048         # memory length
T = 256          # query length
N = 512          # basis count
H = 8            # heads
D = 64           # head dim
P_GRID = 1000    # integration points
RIDGE_C = 4.5    # F F^T diag (4.0) + ridge (0.5)
W_LAST = 1.0 / 1998.0

N_WARMUP_MM = 9

_CACHE = {}


def _host_constants(Wk, Wv):
    """Fold pooling normalization (1/4.5) and query scale (1/8) into the
    projection weights; build the per-basis quadrature-mass tile."""
    import ml_dtypes
    wk = (Wk.astype(np.float64) / (RIDGE_C * 8.0)).astype(ml_dtypes.bfloat16)
    wv = (Wv.astype(np.float64) / RIDGE_C).astype(ml_dtypes.bfloat16)
    p = np.arange(P_GRID)
    nmap = (512 * p) // 999
    w = np.full(P_GRID, 1.0 / 999.0)
    w[0] = w[-1] = 1.0 / 1998.0
    Wn = np.zeros(N)
    for i in range(P_GRID - 1):
        Wn[nmap[i]] += w[i]
    # win8[p, m*8 + h] = Wn[m*128 + p]  (8 identical cols per n-block m)
    win8 = np.repeat(Wn.reshape(4, 128, 1), 8, axis=2)  # [4, 128, 8]
    win8 = np.ascontiguousarray(win8.transpose(1, 0, 2).reshape(128, 32)
                                ).astype(np.float32)
    return wk, wv, win8


def _build_program():
    import concourse.bass as bass
    import concourse.mybir as mybir
    import concourse.tile as tile
    from concourse import bacc

    f32 = mybir.dt.float32
    bf16 = mybir.dt.bfloat16

    nc = bacc.Bacc(
        "TRN2",
        target_bir_lowering=False,
        debug=False,
        enable_asserts=False,
        num_devices=N_CORES,
    )

    kp_d = nc.dram_tensor("kp", [B_PER, E, L], bf16, kind="ExternalInput").ap()
    qt_d = nc.dram_tensor("qt", [B_PER, E, T], bf16, kind="ExternalInput").ap()
    wk_d = nc.dram_tensor("wk", [E, E], bf16, kind="ExternalInput").ap()
    wv_d = nc.dram_tensor("wv", [E, E], bf16, kind="ExternalInput").ap()
    win8_d = nc.dram_tensor("win8", [128, 32], f32, kind="ExternalInput").ap()
    out_d = nc.dram_tensor("out", [B_PER, T, E], bf16, kind="ExternalOutput").ap()

    from contextlib import ExitStack
    with tile.TileContext(nc) as tc, ExitStack() as ctx:
        _kernel_body(ctx, tc, nc, mybir, kp_d, qt_d, wk_d, wv_d, win8_d, out_d)

    nc.compile()
    return nc


def _kernel_body(ctx, tc, nc, mybir, kp_d, qt_d, wk_d, wv_d, win8_d, out_d):
    f32 = mybir.dt.float32
    bf16 = mybir.dt.bfloat16
    Exp = mybir.ActivationFunctionType.Exp

    def pool(name, bufs, space="SBUF"):
        return ctx.enter_context(tc.tile_pool(name=name, bufs=bufs, space=space))

    consts = pool("consts", 1)
    kpool = pool("kpool", 6)
    t1pool = pool("t1pool", 4)
    plpool = pool("plpool", 8)
    qtpool = pool("qtpool", 8)
    ktpool = pool("ktpool", 8)
    vpool = pool("vpool", 8)
    upool = pool("upool", 4)
    zpool = pool("zpool", 8)
    opool = pool("opool", 4)

    ps_proj = pool("ps_proj", 2, "PSUM")   # [128,512] : 1 bank each
    ps_s = pool("ps_s", 2, "PSUM")         # [128,1024]: 2 banks each
    ps_c = pool("ps_c", 2, "PSUM")         # [128,260] : 1 bank each

    # ---- DMA emission order == ring order: batch-0 k first on both rings
    # (wk ahead of it on sync since the first projections need it), then
    # batch-0 q, then batch-1 k / q.  Transfers drain the rings in order at
    # the shared ~360 GB/s HBM cap, so this ordering sets when compute can
    # start.  Pooling adds are emitted right after each k tile's DMA.
    wk_sb = consts.tile([128, 4 * 512], bf16, tag="wk")   # [e%128, (e//128)*512 + e']
    wv_sb = consts.tile([128, 4 * 512], bf16, tag="wv")
    win8_sb = consts.tile([128, 32], f32, tag="win8")
    qT_b = [[qtpool.tile([128, T], bf16, tag="qT", name=f"qt{b}_{eb}")
             for eb in range(4)] for b in range(B_PER)]
    kt_b = [[kpool.tile([128, L], bf16, tag="k", name=f"k{b}_{et}")
             for et in range(4)] for b in range(B_PER)]
    pooled_b = [[plpool.tile([128, N], bf16, tag="pl", name=f"pl{b}_{kk}")
                 for kk in range(4)] for b in range(B_PER)]

    nc.sync.dma_start(wk_sb[:].rearrange("p (kk e) -> p kk e", kk=4),
                      wk_d.rearrange("(kk p) e -> p kk e", p=128))

    def emit_k(b, et):
        kt = kt_b[b][et]
        dma_eng = nc.sync if et % 2 == 0 else nc.scalar
        dma_eng.dma_start(kt[:], kp_d[b, et * 128:(et + 1) * 128, :])
        # kp is host-permuted so both pooling stages are contiguous-half
        # adds: kp[b, e, i] = k[b, e, 4*(i%512) + i//512], hence
        #   t1[c] = kp[c] + kp[c+1024]; pooled[n] = t1[n] + t1[n+512]
        eng = nc.vector if et < 2 else nc.gpsimd
        t1 = t1pool.tile([128, L // 2], bf16,
                         tag="t1v" if et < 2 else "t1g", name=f"t1_{b}_{et}")
        eng.tensor_add(t1[:], kt[:, 0:1024], kt[:, 1024:2048])
        eng.tensor_add(pooled_b[b][et][:], t1[:, 0:512], t1[:, 512:1024])

    def emit_q(b):
        for eb in range(4):
            eng = nc.sync if eb % 2 == 0 else nc.scalar
            eng.dma_start(qT_b[b][eb][:], qt_d[b, eb * 128:(eb + 1) * 128, :])

    for et in range(4):
        emit_k(0, et)
    nc.scalar.dma_start(wv_sb[:].rearrange("p (kk e) -> p kk e", kk=4),
                        wv_d.rearrange("(kk p) e -> p kk e", p=128))
    nc.sync.dma_start(win8_sb[:], win8_d[:])
    emit_q(0)
    for et in range(4):
        emit_k(1, et)
    emit_q(1)

    # ---- PE warm-up: dummy matmuls with no deps keep HAM busy until the
    # first projection matmuls become ready ----
    dummy = consts.tile([128, 512], bf16, tag="dummy")
    nc.vector.memset(dummy[:], 0.125)
    ps_w = ps_s.tile([128, 1024], f32, tag="ps_s")
    for i in range(N_WARMUP_MM):
        nc.tensor.matmul(ps_w[:, 0:512], dummy[:, 0:128], dummy[:],
                         start=True, stop=True, skip_group_check=True)

    for b in range(B_PER):
        pooled = pooled_b[b]
        qT = qT_b[b]

        # ---- keysT = wk^T @ pooled  -> [e' (4x128 part), n=512] ----
        keysT = []
        for m in range(4):
            ps = ps_proj.tile([128, 512], f32, tag="ps_proj")
            for kk in range(4):
                nc.tensor.matmul(
                    ps[:],
                    wk_sb[:, kk * 512 + m * 128: kk * 512 + (m + 1) * 128],
                    pooled[kk][:],
                    start=(kk == 0), stop=(kk == 3),
                )
            kt_sb = ktpool.tile([128, 512], bf16, tag="keysT")
            # b0 drains fit in scalar's pre-exp idle window; b1 on vector
            if b == 0:
                nc.scalar.copy(kt_sb[:], ps[:])
            else:
                nc.vector.tensor_copy(kt_sb[:], ps[:])
            keysT.append(kt_sb)

        # ---- values' = Wn * (pooled^T @ wv) -> [n (4x128 part), 8*(64+Wn)] ----
        values = []
        for m in range(4):
            ps = ps_proj.tile([128, 512], f32, tag="ps_proj")
            for kk in range(4):
                nc.tensor.matmul(
                    ps[:],
                    pooled[kk][:, m * 128:(m + 1) * 128],
                    wv_sb[:, kk * 512:(kk + 1) * 512],
                    start=(kk == 0), stop=(kk == 3),
                )
            v_sb = vpool.tile([128, 8 * 66], bf16, tag="values")
            vv = v_sb[:].rearrange("p (h c) -> p h c", c=66)
            if b == 0:
                nc.scalar.activation(
                    vv[:, :, 0:64],
                    ps[:].rearrange("p (h d) -> p h d", d=64),
                    mybir.ActivationFunctionType.Copy,
                    scale=win8_sb[:, m * 8:m * 8 + 1],
                )
            else:
                nc.vector.tensor_scalar_mul(
                    vv[:, :, 0:64],
                    ps[:].rearrange("p (h d) -> p h d", d=64),
                    win8_sb[:, m * 8:m * 8 + 1],
                )
            nc.vector.tensor_copy(vv[:, :, 64], win8_sb[:, m * 8:(m + 1) * 8])
            values.append(v_sb)

        # ---- per head pair: scores -> exp -> ctx -> normalized out ----
        out_sbs = [opool.tile([128, E], bf16, tag="out", name=f"out{b}_{mb}")
                   for mb in range(2)]
        for hp in range(4):
            # scores: per h01 one [128, 4nb*256t] PSUM tile (2 banks);
            # the h01 pair runs concurrently via PE row tiling (base
            # partitions 0/64) and drains to different banks.
            ps_h = [ps_s.tile([128, 1024], f32, tag="ps_s",
                              name=f"ps_s{b}_{hp}_{h01}") for h01 in range(2)]
            for nb in range(4):
                for h01 in range(2):
                    nc.tensor.matmul(
                        ps_h[h01][:, nb * 256:(nb + 1) * 256],
                        keysT[hp][h01 * 64:(h01 + 1) * 64,
                                  nb * 128:(nb + 1) * 128],
                        qT[hp][h01 * 64:(h01 + 1) * 64, :],
                        start=True, stop=True,
                        skip_group_check=True,
                    )
            # exp: one big ACTIVATE per h01 (no bias; Wn lives in values')
            u_h = []
            for h01 in range(2):
                u = upool.tile([128, 1024], bf16, tag="u")
                nc.scalar.activation(u[:], ps_h[h01][:], Exp)
                u_h.append(u)

            # ctx (+Z at col 64 of each 65-block): 4 chains into one bank
            psc = ps_c.tile([128, 260], f32, tag="ps_c")
            for h01 in range(2):
                h = hp * 2 + h01
                for mb in range(2):
                    c = (2 * h01 + mb) * 65
                    for nb in range(4):
                        nc.tensor.matmul(
                            psc[:, c:c + 65],
                            u_h[h01][:, nb * 256 + mb * 128:
                                     nb * 256 + (mb + 1) * 128],
                            values[nb][:, h * 66:h * 66 + 65],
                            start=(nb == 0), stop=(nb == 3),
                            skip_group_check=True,
                        )
            # batched Z normalization for the whole head pair; drain-muls
            # read PSUM directly (vector, one on scalar for balance)
            pv = psc[:].rearrange("p (g c) -> p g c", c=65)
            z = zpool.tile([128, 4], f32, tag="z")
            nc.vector.tensor_scalar_add(z[:], pv[:, :, 64], W_LAST)
            zi = zpool.tile([128, 4], f32, tag="zi")
            nc.vector.reciprocal(zi[:], z[:])
            for h01 in range(2):
                h = hp * 2 + h01
                for mb in range(2):
                    idx = 2 * h01 + mb
                    if h01 == 1 and mb == 1:
                        nc.scalar.activation(
                            out_sbs[mb][:, h * 64:(h + 1) * 64],
                            pv[:, idx, 0:64],
                            mybir.ActivationFunctionType.Copy,
                            scale=zi[:, idx:idx + 1],
                        )
                    else:
                        nc.vector.tensor_scalar_mul(
                            out_sbs[mb][:, h * 64:(h + 1) * 64],
                            pv[:, idx, 0:64],
                            zi[:, idx:idx + 1],
                        )
        nc.sync.dma_start(out_d[b, 0:128, :], out_sbs[0][:])
        nc.scalar.dma_start(out_d[b, 128:256, :], out_sbs[1][:])


def _get_program():
    if "nc" not in _CACHE:
        _CACHE["nc"] = _build_program()
    return _CACHE["nc"]


def make_in_maps(k, q, Wk, Wv):
    import ml_dtypes
    wk, wv, win8 = _host_constants(Wk, Wv)
    k16 = np.asarray(k).astype(ml_dtypes.bfloat16)
    # kp[b, e, i] = k[b, e, 4*(i%512) + i//512]  (makes both pooling
    # stages contiguous-half adds on device)
    kp = np.ascontiguousarray(
        k16.reshape(B_FULL, E, N, 4).transpose(0, 1, 3, 2).reshape(B_FULL, E, L))
    qt = np.ascontiguousarray(
        np.asarray(q).astype(ml_dtypes.bfloat16).transpose(0, 2, 1))
    in_maps = []
    for c in range(N_CORES):
        in_maps.append({
            "kp": np.ascontiguousarray(kp[c * B_PER:(c + 1) * B_PER]),
            "qt": np.ascontiguousarray(qt[c * B_PER:(c + 1) * B_PER]),
            "wk": wk,
            "wv": wv,
            "win8": win8,
        })
    return in_maps


def kernel(k, q, Wk, Wv):
    from concourse.bass_utils import run_bass_kernel_spmd

    in_maps = make_in_maps(k, q, Wk, Wv)
    nc = _get_program()
    res = run_bass_kernel_spmd(nc, in_maps, core_ids=list(range(N_CORES)))
    return np.concatenate(
        [res.results[c]["out"].astype(np.float32) for c in range(N_CORES)],
        axis=0)
